# revision 27
# baseline (speedup 1.0000x reference)
"""Trainium2 Bass kernel for nn_CrossAAttn_adalora_sym_m (windowed cross-modal
attention with AdaLoRA 1x1 convs, depthwise-7x7 positional conv, BN folding).

Sharding: 8 cores = 2 modalities x 4 batches, fully SPMD (no collectives).
Each core processes one (modality, batch): x_q/x_kv (256, 4096) -> y (256, 4096).

All LoRA deltas and BatchNorms are folded into weights/biases on the host.
On-chip per core:
  Qf = Wq@xq + bq ; Kf = Wk@xkv + bk ; Vn = Wv@xkv + bv (head-major rows)
  Vt = xkv^T @ Wv^T + bv  (token-major, for the o-matmul)
  per (window, head): sT = Kf_h^T Qf_h ; e = exp(sT/sqrt(32)) (ACT engine)
     o = Vt_h^T e (PE, 4-head col-tiled) ; den = ones^T e (PE) ; o /= den
  pe = depthwise7x7(Vn)  via PE 16-tile-packed 32x32 diagonal matmuls
  y = Wp@(o + pe) + btot  (pe partials consumed with rearranged Wp slices)
"""

import sys

sys.path.insert(0, "/opt/trn_rl_repo")

import numpy as np

import concourse.bass as bass
import concourse.mybir as mybir
import concourse.tile as tile
from concourse import bacc
from concourse.bass_utils import run_bass_kernel_spmd

F32 = mybir.dt.float32

DIM, HEADS, HD, AREA = 256, 8, 32, 4
SCALING, EPS = 2.0, 1e-5
B, H, W = 4, 64, 64
N, NW = H * W, H * W // AREA  # 4096, 1024
ISQ = float(1.0 / np.sqrt(HD))
NCORES = 8


# --------------------------------------------------------------------------
# host-side weight prep
# --------------------------------------------------------------------------

def _prep_mod(params, mod):
    def lora_w(p):
        w = np.asarray(p["w"], np.float32)
        A = np.asarray(p["A_" + mod], np.float32)
        Bm = np.asarray(p["B_" + mod], np.float32)
        E = np.asarray(p["E"], np.float32)
        return w + (Bm @ (A * E)).reshape(w.shape) * SCALING

    def bn_fold(p):
        g = np.asarray(p["g_" + mod], np.float32)
        b = np.asarray(p["b_" + mod], np.float32)
        m = np.asarray(p["m_" + mod], np.float32)
        v = np.asarray(p["v_" + mod], np.float32)
        s = g / np.sqrt(v + EPS)
        return s, b - m * s

    Wq = lora_w(params["q"])[:, :, 0, 0]
    sq, bq = bn_fold(params["q"])
    Wqf, bqf = Wq * sq[:, None], bq

    Wkv = lora_w(params["kv"])[:, :, 0, 0]
    skv, bkv = bn_fold(params["kv"])
    Wkvf, bkvf = Wkv * skv[:, None], bkv
    hh = np.arange(256) // 32
    dd = np.arange(256) % 32
    Wk, bk = Wkvf[64 * hh + dd], bkvf[64 * hh + dd]
    Wv, bv = Wkvf[64 * hh + 32 + dd], bkvf[64 * hh + 32 + dd]

    Wpe = lora_w(params["pe"])  # (256,1,7,7)
    spe, bpe = bn_fold(params["pe"])
    wpe = Wpe[:, 0] * spe[:, None, None]  # (256,7,7)

    Wp = lora_w(params["proj"])[:, :, 0, 0]
    sp, bp = bn_fold(params["proj"])
    Wpf = Wp * sp[:, None]
    btot = bp + Wpf @ bpe

    # ---- pack into on-chip layouts ----
    def pack_lhsT_2x2(Wf):  # (128, 512): [p, kt*256 + mt*128 + m] = Wf[128mt+m, 128kt+p]
        arr = np.zeros((128, 512), np.float32)
        for kt in range(2):
            for mt in range(2):
                arr[:, kt * 256 + mt * 128:kt * 256 + mt * 128 + 128] = \
                    Wf[128 * mt:128 * mt + 128, 128 * kt:128 * kt + 128].T
        return arr

    wq_p = pack_lhsT_2x2(Wqf)
    wk_p = pack_lhsT_2x2(Wk)
    wv_p = pack_lhsT_2x2(Wv)
    wpo_p = pack_lhsT_2x2(Wpf)

    # transposed-conv rhs for Vt: (128, 512): [p, kt*256 + d] = Wv[d, 128kt+p]
    wvt_p = np.zeros((128, 512), np.float32)
    for kt in range(2):
        wvt_p[:, kt * 256:kt * 256 + 256] = Wv[:, 128 * kt:128 * kt + 128].T

    # depthwise diagonal tiles: (2, 128, 1568): [vt][32i+p, 32t+p] = wpe[128vt+32i+p, t]
    diag_p = np.zeros((2, 128, 49 * 32), np.float32)
    for vt in range(2):
        for t in range(49):
            blk = diag_p[vt, :, 32 * t:32 * t + 32]
            for i in range(4):
                np.fill_diagonal(blk[32 * i:32 * i + 32, :],
                                 wpe[128 * vt + 32 * i:128 * vt + 32 * i + 32, t // 7, t % 7])

    # proj lhsT slices for the depthwise partials, replicated at all 4
    # partition blocks: (128, 2048): [32j+d, ((vt*4+i)*2+mt)*128+m] = Wpf[128mt+m, 128vt+32i+d]
    wpp_p = np.zeros((128, 2048), np.float32)
    for vt in range(2):
        for i in range(4):
            for mt in range(2):
                sl = Wpf[128 * mt:128 * mt + 128,
                         128 * vt + 32 * i:128 * vt + 32 * i + 32].T  # (32,128)
                col = ((vt * 4 + i) * 2 + mt) * 128
                for j in range(4):
                    wpp_p[32 * j:32 * j + 32, col:col + 128] = sl

    def b2(bvec):  # (128, 2): [p, mt] = bvec[128mt+p]
        return np.stack([bvec[:128], bvec[128:]], axis=1).astype(np.float32)

    return {
        "wq": wq_p, "wk": wk_p, "wv": wv_p, "wvt": wvt_p,
        "wpo": wpo_p, "wpp": wpp_p,
        "diag0": diag_p[0], "diag1": diag_p[1],
        "bq": b2(bqf), "bk": b2(bk), "bvn": b2(bv),
        "bvrow": bv.reshape(1, 256).astype(np.float32),
        "btot": b2(btot),
    }


# --------------------------------------------------------------------------
# bass program (identical for all 8 cores)
# --------------------------------------------------------------------------

_NC_CACHE = {}


def _build_nc(phase=None):
    import os
    if phase is None:
        phase = os.environ.get("BASS_KERNEL_PHASE", "full")
    if phase in _NC_CACHE:
        return _NC_CACHE[phase]
    nc = bacc.Bacc(None)

    xq_d = nc.declare_dram_parameter("xq", [256, N], F32, isOutput=False)
    xkv_d = nc.declare_dram_parameter("xkv", [256, N], F32, isOutput=False)
    wq_d = nc.declare_dram_parameter("wq", [128, 512], F32, isOutput=False)
    wk_d = nc.declare_dram_parameter("wk", [128, 512], F32, isOutput=False)
    wv_d = nc.declare_dram_parameter("wv", [128, 512], F32, isOutput=False)
    wvt_d = nc.declare_dram_parameter("wvt", [128, 512], F32, isOutput=False)
    wpo_d = nc.declare_dram_parameter("wpo", [128, 512], F32, isOutput=False)
    wpp_d = nc.declare_dram_parameter("wpp", [128, 2048], F32, isOutput=False)
    diag0_d = nc.declare_dram_parameter("diag0", [128, 1568], F32, isOutput=False)
    diag1_d = nc.declare_dram_parameter("diag1", [128, 1568], F32, isOutput=False)
    bq_d = nc.declare_dram_parameter("bq", [128, 2], F32, isOutput=False)
    bk_d = nc.declare_dram_parameter("bk", [128, 2], F32, isOutput=False)
    bvn_d = nc.declare_dram_parameter("bvn", [128, 2], F32, isOutput=False)
    bvrow_d = nc.declare_dram_parameter("bvrow", [1, 256], F32, isOutput=False)
    btot_d = nc.declare_dram_parameter("btot", [128, 2], F32, isOutput=False)
    out_d = nc.declare_dram_parameter("out", [256, N], F32, isOutput=True)

    with tile.TileContext(nc) as tc:
        with tc.tile_pool(name="singles", bufs=1) as singles, \
             tc.tile_pool(name="winp", bufs=2) as winp:

            # ---- constants / weights into SBUF ----
            wq_sb = singles.tile([128, 512], F32)
            wk_sb = singles.tile([128, 512], F32)
            wv_sb = singles.tile([128, 512], F32)
            wvt_sb = singles.tile([128, 512], F32)
            wpo_sb = singles.tile([128, 512], F32)
            wpp_sb = singles.tile([128, 2048], F32)
            diag_sb = [singles.tile([128, 1568], F32, name=f"diag{v}_sb")
                       for v in range(2)]
            bq_sb = singles.tile([128, 2], F32)
            bk_sb = singles.tile([128, 2], F32)
            bvn_sb = singles.tile([128, 2], F32)
            bvrow_sb = singles.tile([128, 256], F32)
            btot_sb = singles.tile([128, 2], F32)
            ones_sb = singles.tile([128, 128], F32)

            for sb, d in ((wq_sb, wq_d), (wk_sb, wk_d), (wv_sb, wv_d),
                          (wvt_sb, wvt_d), (wpo_sb, wpo_d), (wpp_sb, wpp_d),
                          (diag_sb[0], diag0_d), (diag_sb[1], diag1_d),
                          (bq_sb, bq_d), (bk_sb, bk_d), (bvn_sb, bvn_d),
                          (btot_sb, btot_d)):
                nc.sync.dma_start(out=sb, in_=d[:, :])
            nc.sync.dma_start(out=bvrow_sb[0:1, :], in_=bvrow_d[:, :])
            nc.vector.memset(ones_sb, 1.0)

            # persistent surfaces
            # padded Vn image: (70 rows + border) x 70 cols per 128-ch tile
            vn_pad = [singles.tile([128, 70, 70], F32, name=f"vnpad{v}")
                      for v in range(2)]
            for v in range(2):
                nc.vector.memset(vn_pad[v], 0.0)
            o_feat = [singles.tile([128, N], F32, name=f"ofeat{g}")
                      for g in range(2)]

            # ============ phase 1: convs + attention, per window ============
            with tc.tile_pool(name="psum1", bufs=1, space="PSUM") as pp1:
                for w in range(4):
                    tok0 = NW * w
                    xq_t = []
                    xkv_t = []
                    for ct in range(2):
                        xq_c = winp.tile([128, NW], F32, tag="xq", bufs=2)
                        nc.sync.dma_start(
                            out=xq_c, in_=xq_d[128 * ct:128 * ct + 128,
                                              tok0:tok0 + NW])
                        xq_t.append(xq_c)
                        xkv_c = winp.tile([128, NW], F32, tag="xkv", bufs=2)
                        nc.sync.dma_start(
                            out=xkv_c, in_=xkv_d[128 * ct:128 * ct + 128,
                                                 tok0:tok0 + NW])
                        xkv_t.append(xkv_c)

                    # --- 1x1 convs (natural layout) ---
                    def conv_nat(w_sb, x_t, bias_sb, dst_fn, tag):
                        for mt in range(2):
                            for ns in range(2):
                                ps = pp1.tile([128, 512], F32, tag="conv",
                                              bufs=2, name="ps_conv")
                                for kt in range(2):
                                    nc.tensor.matmul(
                                        ps,
                                        lhsT=w_sb[:, kt * 256 + mt * 128:
                                                  kt * 256 + mt * 128 + 128],
                                        rhs=x_t[kt][:, 512 * ns:512 * ns + 512],
                                        start=(kt == 0), stop=(kt == 1))
                                nc.vector.tensor_scalar_add(
                                    dst_fn(mt, ns), ps, bias_sb[:, mt:mt + 1])

                    qf_t = [winp.tile([128, NW], F32, tag="qf", name=f"qf{g}")
                            for g in range(2)]
                    kf_t = [winp.tile([128, NW], F32, tag="kf", name=f"kf{g}")
                            for g in range(2)]
                    conv_nat(wq_sb, xq_t, bq_sb,
                             lambda mt, ns: qf_t[mt][:, 512 * ns:512 * ns + 512], "qf")
                    conv_nat(wk_sb, xkv_t, bk_sb,
                             lambda mt, ns: kf_t[mt][:, 512 * ns:512 * ns + 512], "kf")
                    # Vn into the padded image buffer (8 rows per 512-chunk)
                    for mt in range(2):
                        for ns in range(2):
                            ps = pp1.tile([128, 512], F32, tag="conv", bufs=2,
                                          name="ps_vn")
                            for kt in range(2):
                                nc.tensor.matmul(
                                    ps,
                                    lhsT=wv_sb[:, kt * 256 + mt * 128:
                                               kt * 256 + mt * 128 + 128],
                                    rhs=xkv_t[kt][:, 512 * ns:512 * ns + 512],
                                    start=(kt == 0), stop=(kt == 1))
                            r0 = 16 * w + 8 * ns + 3
                            nc.vector.tensor_scalar_add(
                                vn_pad[mt][:, r0:r0 + 8, 3:67], ps,
                                bvn_sb[:, mt:mt + 1])

                    if phase == "conv":
                        for mt in range(2):
                            nc.sync.dma_start(
                                out=out_d[128 * mt:128 * mt + 128,
                                          tok0:tok0 + NW],
                                in_=qf_t[mt])
                        continue

                    # --- transposed conv: Vt (token-major) ---
                    vt_t = winp.tile([128, 8, 256], F32, tag="vt", bufs=1)
                    for m in range(8):
                        psv = pp1.tile([128, 256], F32, tag="conv", bufs=2,
                                       name="ps_vt")
                        for kt in range(2):
                            nc.tensor.matmul(
                                psv,
                                lhsT=xkv_t[kt][:, 128 * m:128 * m + 128],
                                rhs=wvt_sb[:, 256 * kt:256 * kt + 256],
                                start=(kt == 0), stop=False)
                        nc.tensor.matmul(
                            psv, lhsT=ones_sb[0:1, 0:128],
                            rhs=bvrow_sb[0:1, 0:256], start=False, stop=True)
                        nc.vector.tensor_copy(vt_t[:, m, :], psv)

                    # --- attention ---
                    for g in range(2):
                        for ns in range(2):
                            o_ps = pp1.tile([128, 512], F32, tag="o", name="o_ps")
                            den_ps = pp1.tile([128, 512], F32, tag="den",
                                              name="den_ps")
                            for m in range(8):
                                s_ps = pp1.tile([128, 2048], F32, tag="s",
                                                name="s_ps")
                                for j in range(4):
                                    nc.tensor.matmul(
                                        s_ps[:, 512 * j:512 * j + 512],
                                        lhsT=kf_t[g][32 * j:32 * j + 32,
                                                     128 * m:128 * m + 128],
                                        rhs=qf_t[g][32 * j:32 * j + 32,
                                                    512 * ns:512 * ns + 512],
                                        start=True, stop=True,
                                        tile_position=(32 * j, 0))
                                e_t = winp.tile([128, 2048], F32, tag="e",
                                                name="e_t")
                                import os as _os
                                nexp = int(_os.environ.get("BASS_EXP_SPLIT", "4"))
                                step = 2048 // nexp
                                for xi in range(nexp):
                                    nc.scalar.activation(
                                        e_t[:, step * xi:step * (xi + 1)],
                                        s_ps[:, step * xi:step * (xi + 1)],
                                        mybir.ActivationFunctionType.Exp,
                                        scale=ISQ)
                                for j in range(4):
                                    nc.tensor.matmul(
                                        o_ps[32 * j:32 * j + 32, :],
                                        lhsT=vt_t[:, m,
                                                  128 * g + 32 * j:
                                                  128 * g + 32 * j + 32],
                                        rhs=e_t[:, 512 * j:512 * j + 512],
                                        start=(m == 0), stop=(m == 7),
                                        tile_position=(0, 32 * j),
                                        skip_group_check=True)
                                    nc.tensor.matmul(
                                        den_ps[32 * j:32 * j + 32, :],
                                        lhsT=ones_sb[:, 0:32],
                                        rhs=e_t[:, 512 * j:512 * j + 512],
                                        start=(m == 0), stop=(m == 7),
                                        tile_position=(0, 32 * j),
                                        skip_group_check=True)
                            rec_t = winp.tile([128, 512], F32, tag="rec",
                                              bufs=1, name="rec_t")
                            den_t = winp.tile([128, 512], F32, tag="den_t",
                                              bufs=1, name="den_t")
                            nc.vector.tensor_copy(den_t, den_ps)
                            nc.vector.reciprocal_approx_fast(
                                out=rec_t, in_=den_t)
                            nc.vector.tensor_mul(
                                o_feat[g][:, tok0 + 512 * ns:
                                          tok0 + 512 * ns + 512],
                                o_ps, rec_t)

            if phase == "attn":
                for g in range(2):
                    nc.sync.dma_start(out=out_d[128 * g:128 * g + 128, :],
                                      in_=o_feat[g])
            # ============ phase 2: depthwise 7x7 + proj ============
            from contextlib import nullcontext
            with (tc.tile_pool(name="psum2", bufs=1, space="PSUM")
                  if phase == "full" else nullcontext()) as pp2:
                for c in (range(8) if phase == "full" else []):  # 512-tok chunks
                    r0 = 8 * c
                    dwp = []
                    for vt in range(2):
                        dw_ps = [pp2.tile([128, 512], F32, tag=f"dw{i}",
                                          name=f"dw_ps{i}") for i in range(4)]
                        for i in range(4):
                            for j in range(4):
                                taps = list(range(j, 49, 4))
                                for ti, t in enumerate(taps):
                                    dy, dx = t // 7, t % 7
                                    nc.tensor.matmul(
                                        dw_ps[i][32 * j:32 * j + 32, :],
                                        lhsT=diag_sb[vt][32 * i:32 * i + 32,
                                                         32 * t:32 * t + 32],
                                        rhs=vn_pad[vt][32 * i:32 * i + 32,
                                                       r0 + dy:r0 + dy + 8,
                                                       dx:dx + 64],
                                        start=(ti == 0),
                                        stop=(ti == len(taps) - 1),
                                        tile_position=(32 * i, 32 * j),
                                        skip_group_check=True)
                        for i in range(4):
                            d_t = winp.tile([128, 512], F32, tag="dwp",
                                            bufs=8, name="d_t")
                            nc.vector.tensor_copy(d_t, dw_ps[i])
                            dwp.append(d_t)

                    for mt in range(2):
                        pj = [pp2.tile([128, 512], F32, tag=f"pj{j}",
                                       name=f"pj{j}") for j in range(4)]
                        # Every MM in a bank's accumulation group must use the
                        # same (K=32, row group j) config — mixing a full-K MM
                        # into the group wedges the PE (HW-verified).
                        for j in range(4):
                            for kt in range(2):
                                nc.tensor.matmul(
                                    pj[j],
                                    lhsT=wpo_sb[32 * j:32 * j + 32,
                                                (kt * 2 + mt) * 128:
                                                (kt * 2 + mt) * 128 + 128],
                                    rhs=o_feat[kt][32 * j:32 * j + 32,
                                                   512 * c:512 * c + 512],
                                    start=(kt == 0), stop=False,
                                    tile_position=(32 * j, 0),
                                    skip_group_check=True)
                            for b8 in range(8):
                                nc.tensor.matmul(
                                    pj[j],
                                    lhsT=wpp_sb[32 * j:32 * j + 32,
                                                (b8 * 2 + mt) * 128:
                                                (b8 * 2 + mt) * 128 + 128],
                                    rhs=dwp[b8][32 * j:32 * j + 32, :],
                                    start=False, stop=(b8 == 7),
                                    tile_position=(32 * j, 0),
                                    skip_group_check=True)
                        # DVE may read at most one PSUM operand per op:
                        # chain (psum + sbuf) adds, folding the bias in.
                        t1 = winp.tile([128, 512], F32, tag="prj_t1", bufs=1,
                                       name="t1")
                        t2 = winp.tile([128, 512], F32, tag="prj_t2", bufs=1,
                                       name="t2")
                        t3 = winp.tile([128, 512], F32, tag="prj_t3", bufs=1,
                                       name="t3")
                        y_t = winp.tile([128, 512], F32, tag="y", bufs=2,
                                        name="y_t")
                        add = mybir.AluOpType.add
                        nc.vector.tensor_scalar_add(
                            t1, pj[0], btot_sb[:, mt:mt + 1])
                        nc.vector.scalar_tensor_tensor(
                            out=t2, in0=pj[1], scalar=0.0, in1=t1,
                            op0=add, op1=add)
                        nc.vector.scalar_tensor_tensor(
                            out=t3, in0=pj[2], scalar=0.0, in1=t2,
                            op0=add, op1=add)
                        nc.vector.scalar_tensor_tensor(
                            out=y_t, in0=pj[3], scalar=0.0, in1=t3,
                            op0=add, op1=add)
                        nc.sync.dma_start(
                            out=out_d[128 * mt:128 * mt + 128,
                                      512 * c:512 * c + 512],
                            in_=y_t)

    nc.finalize()
    _NC_CACHE["nc"] = nc
    return nc


# --------------------------------------------------------------------------
# entry point
# --------------------------------------------------------------------------

TRACE = False
LAST = {}


def kernel(q_rgb, q_ir, kv_rgb, kv_ir, params):
    nc = _build_nc()

    packs = {m: _prep_mod(params, m) for m in ("rgb", "ir")}
    xs = {"rgb": (np.asarray(q_rgb, np.float32), np.asarray(kv_rgb, np.float32)),
          "ir": (np.asarray(q_ir, np.float32), np.asarray(kv_ir, np.float32))}

    in_maps = []
    for core in range(NCORES):
        mod = "rgb" if core < 4 else "ir"
        b = core % 4
        pk = packs[mod]
        xq, xkv = xs[mod]
        in_maps.append({
            "xq": np.ascontiguousarray(xq[b].reshape(256, N)),
            "xkv": np.ascontiguousarray(xkv[b].reshape(256, N)),
            **{k: pk[k] for k in ("wq", "wk", "wv", "wvt", "wpo", "wpp",
                                   "diag0", "diag1", "bq", "bk", "bvn",
                                   "bvrow", "btot")},
        })

    res = run_bass_kernel_spmd(nc, in_maps, list(range(NCORES)), trace=TRACE)
    LAST["results"] = res
    LAST["in_maps"] = in_maps

    yr = np.stack([res.results[b]["out"].reshape(256, H, W) for b in range(4)])
    yi = np.stack([res.results[4 + b]["out"].reshape(256, H, W) for b in range(4)])
    return yr.astype(np.float32), yi.astype(np.float32)
